# revision 27
# baseline (speedup 1.0000x reference)
"""Trainium2 Bass kernel for DirCFConv-style GNN message passing.

Computes, for inputs s:(B,N,H) f32, ef_mask:(B,N,N,H) f32, W:(H,H), b:(H,):
    m   = SiLU(LayerNorm(s @ W.T + b))          # (B,N,H)
    out[b,i,h] = sum_j ef_mask[b,i,j,h] * m[b,j,h]

Sharding: 8 cores, core c handles batch b = c // 2 and query-node half
i in [ (c%2)*256, (c%2)*256+256 ).

Per core the mask shard is uploaded as fp16 in a [k, p, jt*ic*h] layout
(host-side transpose during sharding): half the HBM bytes of the f32
original, and every i-chunk DMA is one fully sequential 2 MiB read into
128 partitions x 16 KiB contiguous lines (~full HBM bandwidth).  All
stage-1 inputs and constants (s, W, identity, ones-row, bias, eps) ride
in ONE packed [128, 1025] f32 tensor loaded with a single DMA, so the
m = SiLU(LN(...)) prologue is ready ~10us in, under the mask prefetch.

Stage 2 streams chunks: the vector engine multiplies each fp16 tile by
a replicated fp16 m tile (2x packed mode), and the tensor engine
column-sum-reduces over the j partition axis with a one-hot-selector
matmul whose output partition k holds chunk k; all chunks accumulate in
one PSUM group (no mid-loop release waits).  Matmul traffic is fp16 so
the PE streams at full rate; PSUM accumulation stays fp32.
"""

import numpy as np

import concourse.bass as bass
import concourse.bacc as bacc
import concourse.tile as tile
from concourse import mybir
from concourse.bass_utils import run_bass_kernel_spmd

B, N, H = 4, 512, 128
P = 128
NJT = N // P          # 4 j-tiles of 128 partitions
IC = 16               # i's per chunk -> 2 MiB fp16 DMAs, PSUM partition k = chunk k
IH = N // 2           # 256 i's per core
N_CORES = 8
LN_EPS = 1e-5
F32 = mybir.dt.float32
F16 = mybir.dt.float16
CH = IC * H           # 2048 columns per (jt, chunk)
MMF = 512             # moving-operand columns per matmul (one PSUM bank)

# packed constants tensor column offsets: [s | W | I | ones | bias | eps]
CW = NJT * H          # 512
CID = CW + H          # 640
CON = CID + P         # 768
CB = CON + P          # 896
CEPS = CB + H         # 1024
C32W = CEPS + 1       # 1025


def build_consts(s_b, W, b):
    """Pack s (one batch), W, identity, ones-row, bias, eps into [P, C32W]."""
    c = np.zeros((P, C32W), np.float32)
    sr = np.asarray(s_b, dtype=np.float32).reshape(NJT, P, H)
    for jt in range(NJT):
        c[:, jt * H:(jt + 1) * H] = sr[jt]
    c[:, CW:CW + H] = np.asarray(W, dtype=np.float32)
    c[:, CID:CID + P] = np.eye(P, dtype=np.float32)
    c[:, CON:CON + P] = 1.0
    c[0, CB:CB + H] = np.asarray(b, dtype=np.float32)
    c[:, CEPS] = LN_EPS
    return c


def shard_mask_full(mask_ih):
    """[i, j, h] f32 -> fp16 [k, p, jt*ic*h] chunk-sequential layout."""
    ih = mask_ih.shape[0]
    m = np.asarray(mask_ih, dtype=np.float16).transpose(1, 0, 2)  # [j, i, h]
    m = (
        m.reshape(NJT, P, ih // IC, IC, H)
        .transpose(2, 1, 0, 3, 4)                                 # [k, p, jt, ic, h]
        .reshape(ih // IC, P, NJT * IC * H)
    )
    return np.ascontiguousarray(m)


def build_nc(ih=IH):
    nc = bacc.Bacc()
    nch = ih // IC        # i-chunks; chunk k lands on PSUM partition k
    c32_d = nc.declare_dram_parameter("c32", [P, C32W], F32, isOutput=False)
    mask_d = nc.declare_dram_parameter(
        "mask", [nch, P, NJT * IC * H], F16, isOutput=False
    )
    out_d = nc.declare_dram_parameter("out", [ih, H], F32, isOutput=True)

    with tile.TileContext(nc) as tc:
        with (
            tc.tile_pool(name="consts", bufs=1) as consts,
            tc.tile_pool(name="small", bufs=4) as small,
            tc.tile_pool(name="loads", bufs=6) as loads,
            tc.tile_pool(name="prods", bufs=2) as prods,
            tc.tile_pool(name="outs", bufs=1) as outs,
        ):
            stage1_psum = tc.tile_pool(name="spsum", bufs=1, space="PSUM")
            spsum = stage1_psum.__enter__()
            # ---- one DMA for every stage-1 input/constant, FIRST on the
            # sync queue: it completes before mask chunk 0 instead of
            # time-sharing SDMA packets with the stream for ~12us ----
            c32 = consts.tile([P, C32W], F32)
            nc.sync.dma_start(out=c32, in_=c32_d[:, :])
            w_sb = c32[:, CW:CW + H]
            ident = c32[:, CID:CID + P]
            ones_row = c32[0:1, CON:CON + P]
            bias_sb = c32[0:1, CB:CB + H]
            eps_t = c32[:, CEPS:CEPS + 1]

            # sel[:, k*nch:(k+1)*nch] is a one-hot stationary operand routing
            # chunk k's column-sum to PSUM partition k (zeros to the others,
            # keeping every matmul's footprint the full [nch, MMF] region).
            # Built on DVE so stage-2 matmuls' waits stay single-engine (DVE).
            sel = consts.tile([P, nch * nch], F16)
            nc.vector.memset(sel, 0.0)
            for k in range(nch):
                nc.vector.memset(sel[:, k * nch + k:k * nch + k + 1], 1.0)

            # Mask stream rides the sync HWDGE queue, whose NX has nothing
            # else to do while it stalls on the ring's in-flight DMA limit;
            # the constants load rides the scalar queue so it is not trapped
            # behind those stalls.  The last chunk is split per-jt so its
            # multiplies overlap the stream tail.
            def issue_mask_load(k):
                mt = loads.tile([P, NJT * CH], F16, name=f"mt{k}", tag="mt")
                if k == nch - 1:
                    for jt in range(NJT):
                        nc.sync.dma_start(
                            out=mt[:, jt * CH:(jt + 1) * CH],
                            in_=mask_d[k][:, jt * CH:(jt + 1) * CH],
                        )
                else:
                    nc.sync.dma_start(out=mt, in_=mask_d[k])
                return mt

            nbuf = min(6, nch)
            pre_mts = [issue_mask_load(k) for k in range(nbuf)]

            # W^T via PE-transpose: (o,h) -> (h,o)
            wT_ps = spsum.tile([H, H], F32)
            nc.tensor.transpose(wT_ps, w_sb, ident)
            wT_sb = consts.tile([H, H], F32)
            nc.scalar.copy(wT_sb, wT_ps)

            # ------------- m = SiLU(LN(s @ W.T + b)) -------------
            # All four s^T blocks share one PSUM bank (one zero-region group);
            # likewise the four h = s@W.T+b blocks.  No PSUM slot rotation ->
            # no extra release waits on any Matmult.
            sT_all = spsum.tile([P, NJT * P], F32)
            h_all = spsum.tile([P, NJT * H], F32)
            for jt in range(NJT):
                nc.tensor.matmul(
                    sT_all[:, jt * P:(jt + 1) * P],
                    lhsT=c32[:, jt * H:(jt + 1) * H],
                    rhs=ident,
                    is_transpose=True,
                    start=(jt == 0),
                    stop=(jt == NJT - 1),
                )
            sT_sb = consts.tile([P, NJT * P], F32)
            nc.scalar.copy(sT_sb, sT_all)
            for jt in range(NJT):
                nc.tensor.matmul(
                    h_all[:, jt * H:(jt + 1) * H],
                    lhsT=sT_sb[:, jt * P:(jt + 1) * P],
                    rhs=wT_sb,
                    start=(jt == 0),
                    stop=False,
                )
                nc.tensor.matmul(
                    h_all[:, jt * H:(jt + 1) * H],
                    lhsT=ones_row,
                    rhs=bias_sb,
                    start=False,
                    stop=(jt == NJT - 1),
                )

            # m_rep[:, jt, r, :] = m[jt*128:(jt+1)*128, :] for every r (IC
            # copies).  Two passes so ACT loads each activation table once
            # (alternating Rsqrt/Sigmoid would reload tables at ~1.3us each).
            m_rep = consts.tile([P, NJT, IC, H], F16)
            xns = []
            for jt in range(NJT):
                h_ps = h_all[:, jt * H:(jt + 1) * H]
                stats = small.tile([P, 6], F32, tag=f"st{jt}")
                nc.vector.bn_stats(stats, h_ps)
                mv = small.tile([P, 2], F32, tag=f"mv{jt}")
                nc.vector.bn_aggr(mv, stats)
                xc = small.tile([P, H], F32, tag=f"xc{jt}")
                nc.vector.tensor_scalar_sub(xc, h_ps, mv[:, 0:1])
                stdv = small.tile([P, 1], F32, tag=f"sd{jt}")
                nc.scalar.activation(
                    stdv, mv[:, 1:2], mybir.ActivationFunctionType.Sqrt, bias=eps_t
                )
                rstd = small.tile([P, 1], F32, tag=f"rs{jt}")
                nc.vector.reciprocal(rstd, stdv)
                xn = small.tile([P, H], F32, tag=f"xn{jt}")
                nc.vector.tensor_scalar_mul(xn, xc, rstd)
                xns.append(xn)
            for jt in range(NJT):
                xn = xns[jt]
                sg = small.tile([P, H], F32, tag=f"sg{jt}")
                nc.scalar.activation(sg, xn, mybir.ActivationFunctionType.Sigmoid)
                nc.vector.tensor_mul(m_rep[:, jt, 0, :], xn, sg)
                rep = 1
                while rep < IC:
                    cnt = min(rep, IC - rep)
                    nc.vector.tensor_copy(
                        m_rep[:, jt, rep:rep + cnt, :], m_rep[:, jt, 0:cnt, :]
                    )
                    rep += cnt

            # stage-1 PSUM pool stays open: releasing it would put release
            # waits on stage-2 Matmults, which walrus cannot encode.
            # ------------- out[i,h] = sum_j mask[j,i,h] * m[j,h] -------------
            # acc_c[k, f] += one-hot-routed column sum of the product tile
            # over the j partition axis.  One accumulation group per PSUM
            # bank spans the whole loop -> no mid-loop release waits.
            opsum_cm = tc.tile_pool(name="opsum", bufs=1, space="PSUM")
            opsum = opsum_cm.__enter__()
            accs = [
                opsum.tile([P, MMF], F32, name=f"acc{c}", tag=f"acc{c}")
                for c in range(CH // MMF)
            ]
            for k in range(nch):
                mt = pre_mts[k] if k < nbuf else issue_mask_load(k)
                # all-16-bit streams: DVE tensor_mul runs in 2x packed mode
                # and the PE streams fp16 moving operands at full rate; DVE
                # computes fp32 internally and PSUM accumulation stays fp32.
                pt = prods.tile([P, NJT * CH], F16)
                if k < nch - 1:
                    # one fused multiply per chunk amortizes the per-op DVE
                    # overhead; DVE is within ~10% of DMA pace
                    nc.vector.tensor_mul(
                        pt, mt, m_rep[:, :, :, :].rearrange("p a b c -> p (a b c)")
                    )
                for jt in range(NJT):
                    if k == nch - 1:
                        # last chunk: per-jt multiplies overlap its per-jt DMAs
                        nc.vector.tensor_mul(
                            pt[:, jt * CH:(jt + 1) * CH],
                            mt[:, jt * CH:(jt + 1) * CH],
                            m_rep[:, jt, :, :].rearrange("p a b -> p (a b)"),
                        )
                    for c in range(CH // MMF):
                        nc.tensor.matmul(
                            accs[c][0:nch, :],
                            lhsT=sel[:, k * nch:(k + 1) * nch],
                            rhs=pt[:, jt * CH + c * MMF:jt * CH + (c + 1) * MMF],
                            start=(k == 0 and jt == 0),
                            stop=(k == nch - 1 and jt == NJT - 1),
                        )
            # epilogue: PSUM partition k, column (i_loc, h) -> out row k*IC+i_loc
            # drains split across ACT and DVE, each bank's out-DMA overlapping
            # the next drain on the (uncongested) scalar queue
            o_sb = outs.tile([nch, CH], F32)
            ipc = MMF // H      # i rows per bank chunk
            out_v = out_d[:, :].rearrange("(k i) h -> k i h", i=IC)
            for c in range(CH // MMF):
                if c % 2 == 0:
                    nc.scalar.copy(o_sb[:, c * MMF:(c + 1) * MMF], accs[c][0:nch, :])
                else:
                    nc.vector.tensor_copy(
                        o_sb[:, c * MMF:(c + 1) * MMF], accs[c][0:nch, :]
                    )
                nc.scalar.dma_start(
                    out=out_v[:, c * ipc:(c + 1) * ipc, :],
                    in_=o_sb[:, c * MMF:(c + 1) * MMF],
                )
            opsum_cm.__exit__(None, None, None)
            stage1_psum.__exit__(None, None, None)
    nc.finalize()
    return nc


_NC_CACHE = {}


def _get_nc():
    key = "main"
    if key not in _NC_CACHE:
        _NC_CACHE[key] = build_nc()
    return _NC_CACHE[key]


def kernel(s, ef_mask, W, b):
    nc = _get_nc()
    in_maps = []
    for c in range(N_CORES):
        bb = c // 2
        half = c % 2
        in_maps.append(
            {
                "c32": build_consts(s[bb], W, b),
                "mask": shard_mask_full(ef_mask[bb, half * IH:(half + 1) * IH]),
            }
        )
    res = run_bass_kernel_spmd(nc, in_maps, list(range(N_CORES))).results
    out = np.empty((B, N, H), dtype=np.float32)
    for c in range(N_CORES):
        bb = c // 2
        half = c % 2
        out[bb, half * IH:(half + 1) * IH] = res[c]["out"]
    return out


# revision 28
# speedup vs baseline: 1.0790x; 1.0790x over previous
"""Trainium2 Bass kernel for DirCFConv-style GNN message passing.

Computes, for inputs s:(B,N,H) f32, ef_mask:(B,N,N,H) f32, W:(H,H), b:(H,):
    m   = SiLU(LayerNorm(s @ W.T + b))          # (B,N,H)
    out[b,i,h] = sum_j ef_mask[b,i,j,h] * m[b,j,h]

Sharding: 8 cores, core c handles batch b = c // 2 and query-node half
i in [ (c%2)*256, (c%2)*256+256 ).

Per core the mask shard is uploaded as fp16 in a [k, p, jt*ic*h] layout
(host-side transpose during sharding): half the HBM bytes of the f32
original, and every i-chunk DMA is one fully sequential 2 MiB read into
128 partitions x 16 KiB contiguous lines (~full HBM bandwidth).  All
stage-1 inputs and constants (s, W, identity, ones-row, bias, eps) ride
in ONE packed [128, 1025] f32 tensor loaded with a single DMA, so the
m = SiLU(LN(...)) prologue is ready ~10us in, under the mask prefetch.

Stage 2 streams chunks: the vector engine multiplies each fp16 tile by
a replicated fp16 m tile (2x packed mode), and the tensor engine
column-sum-reduces over the j partition axis with a one-hot-selector
matmul whose output partition k holds chunk k; all chunks accumulate in
one PSUM group (no mid-loop release waits).  Matmul traffic is fp16 so
the PE streams at full rate; PSUM accumulation stays fp32.
"""

import numpy as np

import concourse.bass as bass
import concourse.bacc as bacc
import concourse.tile as tile
from concourse import mybir
from concourse.bass_utils import run_bass_kernel_spmd

B, N, H = 4, 512, 128
P = 128
NJT = N // P          # 4 j-tiles of 128 partitions
IC = 16               # i's per chunk -> 2 MiB fp16 DMAs, PSUM partition k = chunk k
IH = N // 2           # 256 i's per core
N_CORES = 8
LN_EPS = 1e-5
F32 = mybir.dt.float32
F16 = mybir.dt.float16
CH = IC * H           # 2048 columns per (jt, chunk)
MMF = 512             # moving-operand columns per matmul (one PSUM bank)

# packed constants tensor column offsets: [s | W | I | ones | bias | eps]
CW = NJT * H          # 512
CID = CW + H          # 640
CON = CID + P         # 768
CB = CON + P          # 896
CEPS = CB + H         # 1024
C32W = CEPS + 1       # 1025


def build_consts(s_b, W, b):
    """Pack s (one batch), W, identity, ones-row, bias, eps into [P, C32W]."""
    c = np.zeros((P, C32W), np.float32)
    sr = np.asarray(s_b, dtype=np.float32).reshape(NJT, P, H)
    for jt in range(NJT):
        c[:, jt * H:(jt + 1) * H] = sr[jt]
    c[:, CW:CW + H] = np.asarray(W, dtype=np.float32)
    c[:, CID:CID + P] = np.eye(P, dtype=np.float32)
    c[:, CON:CON + P] = 1.0
    c[0, CB:CB + H] = np.asarray(b, dtype=np.float32)
    c[:, CEPS] = LN_EPS
    return c


def shard_mask_full(mask_ih):
    """[i, j, h] f32 -> fp16 [k, p, jt*ic*h] chunk-sequential layout."""
    ih = mask_ih.shape[0]
    m = np.asarray(mask_ih, dtype=np.float16).transpose(1, 0, 2)  # [j, i, h]
    m = (
        m.reshape(NJT, P, ih // IC, IC, H)
        .transpose(2, 1, 0, 3, 4)                                 # [k, p, jt, ic, h]
        .reshape(ih // IC, P, NJT * IC * H)
    )
    return np.ascontiguousarray(m)


def build_nc(ih=IH):
    nc = bacc.Bacc()
    nch = ih // IC        # i-chunks; chunk k lands on PSUM partition k
    c32_d = nc.declare_dram_parameter("c32", [P, C32W], F32, isOutput=False)
    mask_d = nc.declare_dram_parameter(
        "mask", [nch, P, NJT * IC * H], F16, isOutput=False
    )
    out_d = nc.declare_dram_parameter("out", [ih, H], F32, isOutput=True)

    with tile.TileContext(nc) as tc:
        with (
            tc.tile_pool(name="consts", bufs=1) as consts,
            tc.tile_pool(name="small", bufs=4) as small,
            tc.tile_pool(name="loads", bufs=6) as loads,
            tc.tile_pool(name="prods", bufs=2) as prods,
            tc.tile_pool(name="outs", bufs=1) as outs,
        ):
            stage1_psum = tc.tile_pool(name="spsum", bufs=1, space="PSUM")
            spsum = stage1_psum.__enter__()
            # ---- one DMA for every stage-1 input/constant, FIRST on the
            # sync queue: it completes before mask chunk 0 instead of
            # time-sharing SDMA packets with the stream for ~12us ----
            c32 = consts.tile([P, C32W], F32)
            nc.sync.dma_start(out=c32, in_=c32_d[:, :])
            w_sb = c32[:, CW:CW + H]
            ident = c32[:, CID:CID + P]
            ones_row = c32[0:1, CON:CON + P]
            bias_sb = c32[0:1, CB:CB + H]
            eps_t = c32[:, CEPS:CEPS + 1]

            # sel[:, k*nch:(k+1)*nch] is a one-hot stationary operand routing
            # chunk k's column-sum to PSUM partition k (zeros to the others,
            # keeping every matmul's footprint the full [nch, MMF] region).
            # Built on DVE so stage-2 matmuls' waits stay single-engine (DVE).
            sel = consts.tile([P, nch * nch], F16)
            nc.vector.memset(sel, 0.0)
            for k in range(nch):
                nc.vector.memset(sel[:, k * nch + k:k * nch + k + 1], 1.0)

            # Mask stream rides the sync HWDGE queue, whose NX has nothing
            # else to do while it stalls on the ring's in-flight DMA limit;
            # the constants load rides the scalar queue so it is not trapped
            # behind those stalls.  The last chunk is split per-jt so its
            # multiplies overlap the stream tail.
            def issue_mask_load(k):
                mt = loads.tile([P, NJT * CH], F16, name=f"mt{k}", tag="mt")
                if k >= nch - 2:
                    for jt in range(NJT):
                        nc.sync.dma_start(
                            out=mt[:, jt * CH:(jt + 1) * CH],
                            in_=mask_d[k][:, jt * CH:(jt + 1) * CH],
                        )
                else:
                    nc.sync.dma_start(out=mt, in_=mask_d[k])
                return mt

            nbuf = min(6, nch)
            pre_mts = [issue_mask_load(k) for k in range(nbuf)]

            # W^T via PE-transpose: (o,h) -> (h,o)
            wT_ps = spsum.tile([H, H], F32)
            nc.tensor.transpose(wT_ps, w_sb, ident)
            wT_sb = consts.tile([H, H], F32)
            nc.scalar.copy(wT_sb, wT_ps)

            # ------------- m = SiLU(LN(s @ W.T + b)) -------------
            # All four s^T blocks share one PSUM bank (one zero-region group);
            # likewise the four h = s@W.T+b blocks.  No PSUM slot rotation ->
            # no extra release waits on any Matmult.
            sT_all = spsum.tile([P, NJT * P], F32)
            h_all = spsum.tile([P, NJT * H], F32)
            for jt in range(NJT):
                nc.tensor.matmul(
                    sT_all[:, jt * P:(jt + 1) * P],
                    lhsT=c32[:, jt * H:(jt + 1) * H],
                    rhs=ident,
                    is_transpose=True,
                    start=(jt == 0),
                    stop=(jt == NJT - 1),
                )
            sT_sb = consts.tile([P, NJT * P], F32)
            nc.scalar.copy(sT_sb, sT_all)
            for jt in range(NJT):
                nc.tensor.matmul(
                    h_all[:, jt * H:(jt + 1) * H],
                    lhsT=sT_sb[:, jt * P:(jt + 1) * P],
                    rhs=wT_sb,
                    start=(jt == 0),
                    stop=False,
                )
                nc.tensor.matmul(
                    h_all[:, jt * H:(jt + 1) * H],
                    lhsT=ones_row,
                    rhs=bias_sb,
                    start=False,
                    stop=(jt == NJT - 1),
                )

            # m_rep[:, jt, r, :] = m[jt*128:(jt+1)*128, :] for every r (IC
            # copies).  Two passes so ACT loads each activation table once
            # (alternating Rsqrt/Sigmoid would reload tables at ~1.3us each).
            m_rep = consts.tile([P, NJT, IC, H], F16)
            xns = []
            for jt in range(NJT):
                h_ps = h_all[:, jt * H:(jt + 1) * H]
                stats = small.tile([P, 6], F32, tag=f"st{jt}")
                nc.vector.bn_stats(stats, h_ps)
                mv = small.tile([P, 2], F32, tag=f"mv{jt}")
                nc.vector.bn_aggr(mv, stats)
                xc = small.tile([P, H], F32, tag=f"xc{jt}")
                nc.vector.tensor_scalar_sub(xc, h_ps, mv[:, 0:1])
                stdv = small.tile([P, 1], F32, tag=f"sd{jt}")
                nc.scalar.activation(
                    stdv, mv[:, 1:2], mybir.ActivationFunctionType.Sqrt, bias=eps_t
                )
                rstd = small.tile([P, 1], F32, tag=f"rs{jt}")
                nc.vector.reciprocal(rstd, stdv)
                xn = small.tile([P, H], F32, tag=f"xn{jt}")
                nc.vector.tensor_scalar_mul(xn, xc, rstd)
                xns.append(xn)
            for jt in range(NJT):
                xn = xns[jt]
                sg = small.tile([P, H], F32, tag=f"sg{jt}")
                nc.scalar.activation(sg, xn, mybir.ActivationFunctionType.Sigmoid)
                nc.vector.tensor_mul(m_rep[:, jt, 0, :], xn, sg)
                rep = 1
                while rep < IC:
                    cnt = min(rep, IC - rep)
                    nc.vector.tensor_copy(
                        m_rep[:, jt, rep:rep + cnt, :], m_rep[:, jt, 0:cnt, :]
                    )
                    rep += cnt

            # stage-1 PSUM pool stays open: releasing it would put release
            # waits on stage-2 Matmults, which walrus cannot encode.
            # ------------- out[i,h] = sum_j mask[j,i,h] * m[j,h] -------------
            # acc_c[k, f] += one-hot-routed column sum of the product tile
            # over the j partition axis.  One accumulation group per PSUM
            # bank spans the whole loop -> no mid-loop release waits.
            opsum_cm = tc.tile_pool(name="opsum", bufs=1, space="PSUM")
            opsum = opsum_cm.__enter__()
            accs = [
                opsum.tile([P, MMF], F32, name=f"acc{c}", tag=f"acc{c}")
                for c in range(CH // MMF)
            ]
            for k in range(nch):
                mt = pre_mts[k] if k < nbuf else issue_mask_load(k)
                # all-16-bit streams: DVE tensor_mul runs in 2x packed mode
                # and the PE streams fp16 moving operands at full rate; DVE
                # computes fp32 internally and PSUM accumulation stays fp32.
                pt = prods.tile([P, NJT * CH], F16)
                if k < nch - 2:
                    # one fused multiply per chunk amortizes the per-op DVE
                    # overhead; DVE is within ~10% of DMA pace
                    nc.vector.tensor_mul(
                        pt, mt, m_rep[:, :, :, :].rearrange("p a b c -> p (a b c)")
                    )
                for jt in range(NJT):
                    if k >= nch - 2:
                        # tail chunks: per-jt multiplies overlap per-jt DMAs
                        nc.vector.tensor_mul(
                            pt[:, jt * CH:(jt + 1) * CH],
                            mt[:, jt * CH:(jt + 1) * CH],
                            m_rep[:, jt, :, :].rearrange("p a b -> p (a b)"),
                        )
                    for c in range(CH // MMF):
                        nc.tensor.matmul(
                            accs[c][0:nch, :],
                            lhsT=sel[:, k * nch:(k + 1) * nch],
                            rhs=pt[:, jt * CH + c * MMF:jt * CH + (c + 1) * MMF],
                            start=(k == 0 and jt == 0),
                            stop=(k == nch - 1 and jt == NJT - 1),
                        )
            # epilogue: PSUM partition k, column (i_loc, h) -> out row k*IC+i_loc
            # drains split across ACT and DVE, then one 128 KiB out-DMA on the
            # (uncongested) scalar queue
            o_sb = outs.tile([nch, CH], F32)
            for c in range(CH // MMF):
                if c % 2 == 0:
                    nc.scalar.copy(o_sb[:, c * MMF:(c + 1) * MMF], accs[c][0:nch, :])
                else:
                    nc.vector.tensor_copy(
                        o_sb[:, c * MMF:(c + 1) * MMF], accs[c][0:nch, :]
                    )
            nc.scalar.dma_start(
                out=out_d[:, :].rearrange("(k i) h -> k (i h)", i=IC), in_=o_sb
            )
            opsum_cm.__exit__(None, None, None)
            stage1_psum.__exit__(None, None, None)
    nc.finalize()
    return nc


_NC_CACHE = {}


def _get_nc():
    key = "main"
    if key not in _NC_CACHE:
        _NC_CACHE[key] = build_nc()
    return _NC_CACHE[key]


def kernel(s, ef_mask, W, b):
    nc = _get_nc()
    in_maps = []
    for c in range(N_CORES):
        bb = c // 2
        half = c % 2
        in_maps.append(
            {
                "c32": build_consts(s[bb], W, b),
                "mask": shard_mask_full(ef_mask[bb, half * IH:(half + 1) * IH]),
            }
        )
    res = run_bass_kernel_spmd(nc, in_maps, list(range(N_CORES))).results
    out = np.empty((B, N, H), dtype=np.float32)
    for c in range(N_CORES):
        bb = c // 2
        half = c % 2
        out[bb, half * IH:(half + 1) * IH] = res[c]["out"]
    return out


# revision 30
# speedup vs baseline: 1.0830x; 1.0038x over previous
"""Trainium2 Bass kernel for DirCFConv-style GNN message passing.

Computes, for inputs s:(B,N,H) f32, ef_mask:(B,N,N,H) f32, W:(H,H), b:(H,):
    m   = SiLU(LayerNorm(s @ W.T + b))          # (B,N,H)
    out[b,i,h] = sum_j ef_mask[b,i,j,h] * m[b,j,h]

Sharding: 8 cores, core c handles batch b = c // 2 and query-node half
i in [ (c%2)*256, (c%2)*256+256 ).

Per core the mask shard is uploaded as fp16 in a [k, p, jt*ic*h] layout
(host-side transpose during sharding): half the HBM bytes of the f32
original, and every i-chunk DMA is one fully sequential 2 MiB read into
128 partitions x 16 KiB contiguous lines (~full HBM bandwidth).  All
stage-1 inputs and constants (s, W, identity, ones-row, bias, eps) ride
in ONE packed [128, 1025] f32 tensor loaded by the first DMA on the
sync queue, so the m = SiLU(LN(...)) prologue completes under the mask
prefetch and the 32 MiB stream runs gap-free at ~350-400 GB/s.

Stage 2 streams chunks: the vector engine multiplies each fp16 tile by
a replicated fp16 m tile (2x packed mode), and the tensor engine
column-sum-reduces over the j partition axis with a one-hot-selector
matmul whose output partition k holds chunk k; all chunks accumulate in
one PSUM group (no mid-loop release waits).  Matmul traffic is fp16 so
the PE streams at full rate; PSUM accumulation stays fp32.
"""

import numpy as np

import concourse.bass as bass
import concourse.bacc as bacc
import concourse.tile as tile
from concourse import mybir
from concourse.bass_utils import run_bass_kernel_spmd

B, N, H = 4, 512, 128
P = 128
NJT = N // P          # 4 j-tiles of 128 partitions
IC = 16               # i's per chunk -> 2 MiB fp16 DMAs, PSUM partition k = chunk k
IH = N // 2           # 256 i's per core
N_CORES = 8
LN_EPS = 1e-5
F32 = mybir.dt.float32
F16 = mybir.dt.float16
CH = IC * H           # 2048 columns per (jt, chunk)
MMF = 512             # moving-operand columns per matmul (one PSUM bank)

# packed constants tensor column offsets: [s | W | I | ones | bias | eps]
CW = NJT * H          # 512
CID = CW + H          # 640
CON = CID + P         # 768
CB = CON + P          # 896
CEPS = CB + H         # 1024
C32W = CEPS + 1       # 1025


def build_consts(s_b, W, b):
    """Pack s (one batch), W, identity, ones-row, bias, eps into [P, C32W]."""
    c = np.zeros((P, C32W), np.float32)
    sr = np.asarray(s_b, dtype=np.float32).reshape(NJT, P, H)
    for jt in range(NJT):
        c[:, jt * H:(jt + 1) * H] = sr[jt]
    c[:, CW:CW + H] = np.asarray(W, dtype=np.float32)
    c[:, CID:CID + P] = np.eye(P, dtype=np.float32)
    c[:, CON:CON + P] = 1.0
    c[0, CB:CB + H] = np.asarray(b, dtype=np.float32)
    c[:, CEPS] = LN_EPS
    return c


def shard_mask_full(mask_ih):
    """[i, j, h] f32 -> fp16 [k, p, jt*ic*h] chunk-sequential layout."""
    ih = mask_ih.shape[0]
    m = np.asarray(mask_ih, dtype=np.float16).transpose(1, 0, 2)  # [j, i, h]
    m = (
        m.reshape(NJT, P, ih // IC, IC, H)
        .transpose(2, 1, 0, 3, 4)                                 # [k, p, jt, ic, h]
        .reshape(ih // IC, P, NJT * IC * H)
    )
    return np.ascontiguousarray(m)


def build_nc(ih=IH):
    nc = bacc.Bacc()
    nch = ih // IC        # i-chunks; chunk k lands on PSUM partition k
    c32_d = nc.declare_dram_parameter("c32", [P, C32W], F32, isOutput=False)
    mask_d = nc.declare_dram_parameter(
        "mask", [nch, P, NJT * IC * H], F16, isOutput=False
    )
    out_d = nc.declare_dram_parameter("out", [ih, H], F32, isOutput=True)

    with tile.TileContext(nc) as tc:
        with (
            tc.tile_pool(name="consts", bufs=1) as consts,
            tc.tile_pool(name="small", bufs=4) as small,
            tc.tile_pool(name="loads", bufs=6) as loads,
            tc.tile_pool(name="prods", bufs=2) as prods,
            tc.tile_pool(name="outs", bufs=1) as outs,
        ):
            stage1_psum = tc.tile_pool(name="spsum", bufs=1, space="PSUM")
            spsum = stage1_psum.__enter__()
            # ---- one DMA for every stage-1 input/constant, FIRST on the
            # sync queue: it completes before mask chunk 0 instead of
            # time-sharing SDMA packets with the stream for ~12us ----
            c32 = consts.tile([P, C32W], F32)
            nc.sync.dma_start(out=c32, in_=c32_d[:, :])
            w_sb = c32[:, CW:CW + H]
            ident = c32[:, CID:CID + P]
            ones_row = c32[0:1, CON:CON + P]
            bias_sb = c32[0:1, CB:CB + H]
            eps_t = c32[:, CEPS:CEPS + 1]

            # sel[:, k*nch:(k+1)*nch] is a one-hot stationary operand routing
            # chunk k's column-sum to PSUM partition k (zeros to the others,
            # keeping every matmul's footprint the full [nch, MMF] region).
            # Built on DVE so stage-2 matmuls' waits stay single-engine (DVE).
            sel = consts.tile([P, nch * nch], F16)
            nc.vector.memset(sel, 0.0)
            for k in range(nch):
                nc.vector.memset(sel[:, k * nch + k:k * nch + k + 1], 1.0)

            # Mask stream rides the sync HWDGE queue behind the constants
            # load; the sync NX has nothing else to do while it stalls on the
            # ring's in-flight DMA limit.  The last two chunks are split
            # per-jt so their multiplies overlap the stream tail.
            def issue_mask_load(k):
                mt = loads.tile([P, NJT * CH], F16, name=f"mt{k}", tag="mt")
                if k >= nch - 2:
                    for jt in range(NJT):
                        nc.sync.dma_start(
                            out=mt[:, jt * CH:(jt + 1) * CH],
                            in_=mask_d[k][:, jt * CH:(jt + 1) * CH],
                        )
                else:
                    nc.sync.dma_start(out=mt, in_=mask_d[k])
                return mt

            nbuf = min(6, nch)
            pre_mts = [issue_mask_load(k) for k in range(nbuf)]

            # W^T via PE-transpose: (o,h) -> (h,o)
            wT_ps = spsum.tile([H, H], F32)
            nc.tensor.transpose(wT_ps, w_sb, ident)
            wT_sb = consts.tile([H, H], F32)
            nc.scalar.copy(wT_sb, wT_ps)

            # ------------- m = SiLU(LN(s @ W.T + b)) -------------
            # All four s^T blocks share one PSUM bank (one zero-region group);
            # likewise the four h = s@W.T+b blocks.  No PSUM slot rotation ->
            # no extra release waits on any Matmult.
            sT_all = spsum.tile([P, NJT * P], F32)
            h_all = spsum.tile([P, NJT * H], F32)
            for jt in range(NJT):
                nc.tensor.matmul(
                    sT_all[:, jt * P:(jt + 1) * P],
                    lhsT=c32[:, jt * H:(jt + 1) * H],
                    rhs=ident,
                    is_transpose=True,
                    start=(jt == 0),
                    stop=(jt == NJT - 1),
                )
            sT_sb = consts.tile([P, NJT * P], F32)
            nc.scalar.copy(sT_sb, sT_all)
            for jt in range(NJT):
                nc.tensor.matmul(
                    h_all[:, jt * H:(jt + 1) * H],
                    lhsT=sT_sb[:, jt * P:(jt + 1) * P],
                    rhs=wT_sb,
                    start=(jt == 0),
                    stop=False,
                )
                nc.tensor.matmul(
                    h_all[:, jt * H:(jt + 1) * H],
                    lhsT=ones_row,
                    rhs=bias_sb,
                    start=False,
                    stop=(jt == NJT - 1),
                )

            # m_rep[:, jt, r, :] = m[jt*128:(jt+1)*128, :] for every r (IC
            # copies).  Two passes so ACT loads each activation table once
            # (alternating Rsqrt/Sigmoid would reload tables at ~1.3us each).
            m_rep = consts.tile([P, NJT, IC, H], F16)
            xns = []
            for jt in range(NJT):
                h_ps = h_all[:, jt * H:(jt + 1) * H]
                stats = small.tile([P, 6], F32, tag=f"st{jt}")
                nc.vector.bn_stats(stats, h_ps)
                mv = small.tile([P, 2], F32, tag=f"mv{jt}")
                nc.vector.bn_aggr(mv, stats)
                xc = small.tile([P, H], F32, tag=f"xc{jt}")
                nc.vector.tensor_scalar_sub(xc, h_ps, mv[:, 0:1])
                stdv = small.tile([P, 1], F32, tag=f"sd{jt}")
                nc.scalar.activation(
                    stdv, mv[:, 1:2], mybir.ActivationFunctionType.Sqrt, bias=eps_t
                )
                rstd = small.tile([P, 1], F32, tag=f"rs{jt}")
                nc.vector.reciprocal(rstd, stdv)
                xn = small.tile([P, H], F32, tag=f"xn{jt}")
                nc.vector.tensor_scalar_mul(xn, xc, rstd)
                xns.append(xn)
            for jt in range(NJT):
                xn = xns[jt]
                sg = small.tile([P, H], F32, tag=f"sg{jt}")
                nc.scalar.activation(sg, xn, mybir.ActivationFunctionType.Sigmoid)
                nc.vector.tensor_mul(m_rep[:, jt, 0, :], xn, sg)
                rep = 1
                while rep < IC:
                    cnt = min(rep, IC - rep)
                    nc.vector.tensor_copy(
                        m_rep[:, jt, rep:rep + cnt, :], m_rep[:, jt, 0:cnt, :]
                    )
                    rep += cnt

            # stage-1 PSUM pool stays open: releasing it would put release
            # waits on stage-2 Matmults, which walrus cannot encode.
            # ------------- out[i,h] = sum_j mask[j,i,h] * m[j,h] -------------
            # acc_c[k, f] += one-hot-routed column sum of the product tile
            # over the j partition axis.  One accumulation group per PSUM
            # bank spans the whole loop -> no mid-loop release waits.
            opsum_cm = tc.tile_pool(name="opsum", bufs=1, space="PSUM")
            opsum = opsum_cm.__enter__()
            accs = [
                opsum.tile([P, MMF], F32, name=f"acc{c}", tag=f"acc{c}")
                for c in range(CH // MMF)
            ]
            for k in range(nch):
                mt = pre_mts[k] if k < nbuf else issue_mask_load(k)
                # all-16-bit streams: DVE tensor_mul runs in 2x packed mode
                # and the PE streams fp16 moving operands at full rate; DVE
                # computes fp32 internally and PSUM accumulation stays fp32.
                pt = prods.tile([P, NJT * CH], F16)
                if k < nch - 2:
                    # one fused multiply per chunk amortizes the per-op DVE
                    # overhead; DVE is within ~10% of DMA pace
                    nc.vector.tensor_mul(
                        pt, mt, m_rep[:, :, :, :].rearrange("p a b c -> p (a b c)")
                    )
                for jt in range(NJT):
                    if k >= nch - 2:
                        # tail chunks: per-jt multiplies overlap per-jt DMAs
                        nc.vector.tensor_mul(
                            pt[:, jt * CH:(jt + 1) * CH],
                            mt[:, jt * CH:(jt + 1) * CH],
                            m_rep[:, jt, :, :].rearrange("p a b -> p (a b)"),
                        )
                    for c in range(CH // MMF):
                        nc.tensor.matmul(
                            accs[c][0:nch, :],
                            lhsT=sel[:, k * nch:(k + 1) * nch],
                            rhs=pt[:, jt * CH + c * MMF:jt * CH + (c + 1) * MMF],
                            start=(k == 0 and jt == 0),
                            stop=(k == nch - 1 and jt == NJT - 1),
                        )
            # epilogue: PSUM partition k, column (i_loc, h) -> out row k*IC+i_loc
            # drains split across ACT and DVE, then one 128 KiB out-DMA on the
            # (uncongested) scalar queue
            o_sb = outs.tile([nch, CH], F32)
            for c in range(CH // MMF):
                if c % 2 == 0:
                    nc.scalar.copy(o_sb[:, c * MMF:(c + 1) * MMF], accs[c][0:nch, :])
                else:
                    nc.vector.tensor_copy(
                        o_sb[:, c * MMF:(c + 1) * MMF], accs[c][0:nch, :]
                    )
            nc.scalar.dma_start(
                out=out_d[:, :].rearrange("(k i) h -> k (i h)", i=IC), in_=o_sb
            )
            opsum_cm.__exit__(None, None, None)
            stage1_psum.__exit__(None, None, None)
    nc.finalize()
    return nc


_NC_CACHE = {}


def _get_nc():
    key = "main"
    if key not in _NC_CACHE:
        _NC_CACHE[key] = build_nc()
    return _NC_CACHE[key]


def kernel(s, ef_mask, W, b):
    nc = _get_nc()
    in_maps = []
    for c in range(N_CORES):
        bb = c // 2
        half = c % 2
        in_maps.append(
            {
                "c32": build_consts(s[bb], W, b),
                "mask": shard_mask_full(ef_mask[bb, half * IH:(half + 1) * IH]),
            }
        )
    res = run_bass_kernel_spmd(nc, in_maps, list(range(N_CORES))).results
    out = np.empty((B, N, H), dtype=np.float32)
    for c in range(N_CORES):
        bb = c // 2
        half = c % 2
        out[bb, half * IH:(half + 1) * IH] = res[c]["out"]
    return out
